# revision 4
# baseline (speedup 1.0000x reference)
"""Trainium2 Bass kernel for a pre-LN transformer block (B=2, S=2048, D=1024,
H=16, d_ff=4096), 8-way tensor-parallel, v2.

Key structure (per core, SPMD over 8 cores):
- heads sharded for attention (2 heads/core over all 4096 tokens)
- logits QK^T row-packed via tile_position (two K=64 head matmuls run
  concurrently in the PE array); one exp ACTIVATE per key-tile covers both
  heads ([128,1024]); AV uses the ones-augmented V (M=65) for softmax sums
- the attention outputs (pre-proj, [128 hd, token] bf16) are exchanged with
  an AllToAll per batch (0.25 MB/core each) instead of an 8 MB post-proj
  ReduceScatter; proj moves after the collective
- token ownership is interleaved: core c owns tokens [256c,256c+256) of
  batch 0 AND batch 1, so the b=0 AllToAll (fired mid-attention) lets every
  core run its proj+LN2+MLP half-pipeline while the b=1 AllToAll flies
- LayerNorm folded into matmuls (augmented -mu row; 1/sigma on eviction,
  computed as exp(-0.5*ln(var+eps)) so the whole kernel uses one ACT table
  set: natural_log_exp_and_others)
- MLP runs per 256-token half, streaming w1/w2 quarters per half
"""

import sys

for _p in ("/opt/trn_rl_repo",):
    if _p not in sys.path:
        sys.path.insert(0, _p)

import numpy as np
import ml_dtypes

B, S, D = 2, 2048, 1024
H, HD = 16, 64
FF = 4 * D
T = B * S
NCORES = 8
TC = T // NCORES  # 512 own tokens per core (256 from each batch)
HTC = TC // 2  # 256
P = 128
KT = D // P  # 8
KA = 9  # augmented k-tiles
DAUG = D + P
EPS = 1e-5
NKT = S // P  # 16 key tiles per sequence
NQC = S // 512  # 4 q-panels of 512 per batch
BF16 = ml_dtypes.bfloat16

_CACHE = {}


def _build_program(has_c1, has_bproj, has_c2, has_b1, has_b2):
    import concourse.mybir as mybir
    import concourse.tile as tile
    from concourse import bacc
    from concourse.masks import make_identity
    from contextlib import ExitStack

    f32 = mybir.dt.float32
    bf16 = mybir.dt.bfloat16
    f8 = mybir.dt.float8e4
    AF = mybir.ActivationFunctionType
    ALU = mybir.AluOpType

    nc = bacc.Bacc(None, target_bir_lowering=False)

    # ---- I/O ----
    # all big tensors arrive pre-shuffled partition-major so every DMA is
    # a contiguous multi-KB line per partition
    x_aug_d = nc.declare_dram_parameter("x_aug", [P, 8 * KA * 512], bf16, isOutput=False)
    x_cb_d = nc.declare_dram_parameter("x_cb", [P, B * KT * HTC], bf16, isOutput=False)
    wqkv_d = nc.declare_dram_parameter("wqkv_aug", [P, KA * 3 * P], bf16, isOutput=False)
    wprojT_d = nc.declare_dram_parameter("wprojT", [P, KT * D], bf16, isOutput=False)
    w1_d = nc.declare_dram_parameter("w1_aug", [P, 4 * KT * (FF // 4)], bf16, isOutput=False)
    w2t_d = nc.declare_dram_parameter("w2t", [P, 4 * (FF // P // 4) * D], bf16, isOutput=False)
    aux_d = nc.declare_dram_parameter("aux", [P, 64], f32, isOutput=False)
    out_d = nc.declare_dram_parameter("out_c", [D, TC], f32, isOutput=True)

    core_ids = list(range(NCORES))

    with tile.TileContext(nc) as tc, ExitStack() as ctx:
        const = ctx.enter_context(tc.tile_pool(name="const", bufs=1))
        dram = ctx.enter_context(tc.tile_pool(name="dram", bufs=1, space="DRAM"))

        ident = const.tile([P, P], bf16)
        make_identity(nc, ident)
        ones128 = const.tile([P, P], bf16)
        nc.any.memset(ones128, 1.0)
        eps_col = const.tile([P, 1], f32)
        nc.any.memset(eps_col, EPS)
        nbias_col = const.tile([P, 1], f32)
        nc.any.memset(nbias_col, -2.5)

        wqkv_sb = const.tile([P, KA, 3 * P], bf16)
        aux_sb = const.tile([P, 64], f32)
        # residual input, split per batch half (b=1 half loads post-attention)
        xcb_sb = const.tile([P, KT, TC], bf16)
        wprojT_sb = const.tile([P, KT, D], bf16)

        # spans the whole program
        lng = ctx.enter_context(tc.tile_pool(name="lng", bufs=1))
        x1 = lng.tile([P, KT, TC], bf16)
        attn_all = lng.tile([P, B, KT, HTC], bf16)
        x1aug = lng.tile([P, KT, TC], bf16)
        h2T = lng.tile([P, FF // P, TC], bf16)
        w1p = ctx.enter_context(tc.tile_pool(name="w1p", bufs=2))

        psA = ctx.enter_context(tc.tile_pool(name="psA", bufs=2, space="PSUM"))

        # collective buffers (internal DRAM)
        a2a_src = [
            dram.tile([NCORES * P, HTC], bf16, tag=f"as{b}", name=f"a2a_src{b}")
            for b in range(B)
        ]
        a2a_dst = [
            dram.tile([NCORES * P, HTC], bf16, tag=f"ad{b}", name=f"a2a_dst{b}")
            for b in range(B)
        ]

        x_aug_r = x_aug_d.rearrange("p (c k t) -> p c k t", c=8, k=KA)
        w1_r = w1_d.rearrange("p (q k f) -> p q k f", q=4, k=KT)
        w2_r = w2t_d.rearrange("p (q j d) -> p q j d", q=4, j=FF // P // 4)

        # ======== attention-phase pools (closed before the MLP tail) ========
        work = ctx.enter_context(tc.tile_pool(name="work", bufs=2))
        attn_stack = ExitStack()
        attsb = attn_stack.enter_context(tc.tile_pool(name="attsb", bufs=1))
        xaug_pool = attn_stack.enter_context(tc.tile_pool(name="xaug", bufs=2))
        r1_pool = attn_stack.enter_context(tc.tile_pool(name="r1", bufs=2))
        lgp = attn_stack.enter_context(tc.tile_pool(name="lgp", bufs=2, space="PSUM"))
        avp = attn_stack.enter_context(tc.tile_pool(name="avp", bufs=2, space="PSUM"))
        etp = attn_stack.enter_context(tc.tile_pool(name="etp", bufs=2))
        epi = attn_stack.enter_context(tc.tile_pool(name="epi", bufs=2))

        qkvT = attsb.tile([P, 3, T], bf16)
        # V for AV in fp8 DoubleRow form: per key-tile PAIR, the two tiles'
        # [V_h | ones] blocks at 80-element alignment per head
        vext = attsb.tile([P, B * NKT // 2, 2, 160], f8)



        xa_tiles = {}
        r1_tiles = {}

        def chunk_dma(tch):
            xa = xaug_pool.tile([P, KA, 512], bf16, tag="xa", name=f"xa{tch}")
            nc.sync.dma_start(xa, x_aug_r[:, tch, :, :])
            xa_tiles[tch] = xa

        def chunk_stats(tch):
            # LN1 stats for token chunk tch; writes the augmented -mu row into
            # xa and leaves 1/sigma in r1_tiles[tch]
            xa = xa_tiles[tch]
            pmu = psA.tile([P, 512], f32, tag="a", name="pmu")
            psq = psA.tile([P, 512], f32, tag="a", name="psq")
            xsqs = {}

            def mk_xsq(kt):
                xsqs[kt] = work.tile([P, 512], bf16, tag="xsq", bufs=4, name="xsq")
                nc.vector.tensor_tensor(
                    xsqs[kt], xa[:, kt, :], xa[:, kt, :], ALU.mult
                )

            mk_xsq(0)
            mk_xsq(1)
            for kt in range(KT):
                if kt + 2 < KT:
                    mk_xsq(kt + 2)
                nc.tensor.matmul(
                    pmu, ones128, xa[:, kt, :], start=(kt == 0), stop=(kt == KT - 1)
                )
                nc.tensor.matmul(
                    psq, ones128, xsqs.pop(kt), start=(kt == 0), stop=(kt == KT - 1)
                )
            m1 = work.tile([P, 512], f32, tag="m1")
            nc.vector.tensor_scalar_mul(m1, pmu, 1.0 / D)
            nc.vector.tensor_scalar_mul(xa[0:1, KT, :], m1[0:1, :], -1.0)
            v1 = work.tile([P, 512], f32, tag="v1")
            nc.vector.tensor_scalar_mul(v1, psq, 1.0 / D)
            m2 = work.tile([P, 512], f32, tag="m2")
            nc.vector.tensor_tensor(m2, m1, m1, ALU.mult)
            nc.vector.tensor_tensor(v1, v1, m2, ALU.subtract)
            # 1/sigma = exp(-0.5 * ln(var + eps)): stays in the
            # natural_log_exp table set alongside the softmax exp
            lnv = work.tile([P, 512], f32, tag="m2")
            nc.scalar.activation(lnv, v1, AF.Ln, bias=eps_col)
            r1b = r1_pool.tile([P, 512], f32, tag="r1b", name=f"r1b{tch}")
            nc.scalar.activation(r1b, lnv, AF.Exp, scale=-0.5)
            r1_tiles[tch] = r1b

        def chunk_qkv(tch, ms):
            xa = xa_tiles[tch]
            r1b = r1_tiles[tch]
            tsl = slice(tch * 512, (tch + 1) * 512)
            for m in ms:
                msl = slice(m * P, (m + 1) * P)
                ps = psA.tile([P, 512], f32, tag="a", name="qkvps")
                for kt in range(KA):
                    nc.tensor.matmul(
                        ps, wqkv_sb[:, kt, msl], xa[:, kt, :],
                        start=(kt == 0), stop=(kt == KA - 1),
                    )
                nc.vector.tensor_tensor(qkvT[:, m, tsl], ps, r1b, ALU.mult)
                if has_c1:
                    nc.vector.tensor_scalar(
                        qkvT[:, m, tsl], qkvT[:, m, tsl],
                        aux_sb[:, 48 + m : 49 + m], None, ALU.add,
                    )

        def chunk_vext(tch):
            # V for chunk tch -> transposed per 128-token tile into vext
            b = tch // 4
            for j in range(4):
                kt = (tch % 4) * 4 + j
                t0 = b * S + kt * P
                pt = psA.tile([P, 512], bf16, tag="a", name="pt")[:, 0:P]
                nc.tensor.transpose(pt, qkvT[:, 2, t0 : t0 + P], ident)
                jj = (b * NKT + kt) // 2
                i = kt % 2
                nc.vector.tensor_copy(vext[:, jj, i, 0:64], pt[:, 0:64])
                nc.vector.tensor_copy(vext[:, jj, i, 80:144], pt[:, 64:128])

        # vext ones columns
        nc.any.memset(vext[:, :, :, 64:65], 1.0)
        nc.any.memset(vext[:, :, :, 144:145], 1.0)

        # ============ phase 1: LN1 + qkv + vext for b=0 =====================
        with nc.named_scope("ln1_qkv_b0"):
            chunk_dma(0)
            nc.sync.dma_start(wqkv_sb, wqkv_d.rearrange("p (k e) -> p k e", k=KA))
            nc.sync.dma_start(aux_sb, aux_d[:])
            nc.sync.dma_start(
                xcb_sb[:, :, 0:HTC],
                x_cb_d.rearrange("p (b k t) -> p b k t", b=B, k=KT)[:, 0, :, :],
            )
            for tch in range(4):
                if tch + 1 < 4:
                    chunk_dma(tch + 1)
                chunk_stats(tch)
                chunk_qkv(tch, [0, 1, 2])
                chunk_vext(tch)

        # b=1 prep quanta, interleaved into the b=0 attention panels
        prep_quanta = []
        for tch in range(4, 8):
            prep_quanta.append(lambda t=tch: (chunk_dma(t), chunk_stats(t)))
            prep_quanta.append(lambda t=tch: chunk_qkv(t, [0, 1]))
            prep_quanta.append(lambda t=tch: (chunk_qkv(t, [2]), chunk_vext(t)))

        # ============ phase 2: attention ====================================
        def epilogue(b, qc, h, avq):
            rs = epi.tile([1, 512], f32, tag="rs", name="rs")
            nc.vector.tensor_copy(rs, avq[64:65, :])
            rc = epi.tile([1, 512], f32, tag="rc", name="rc")
            nc.vector.reciprocal_approx_fast(rc, rs)
            rcb = epi.tile([1, 512], bf16, tag="rcb", name="rcb")
            nc.vector.tensor_copy(rcb, rc)
            rbs = epi.tile([64, 512], bf16, tag="rbs", name="rbs")
            nc.gpsimd.partition_broadcast(rbs, rcb, channels=64)
            att = epi.tile([64, 512], bf16, tag="att", name="att")
            nc.vector.tensor_tensor(att, avq[0:64, :], rbs, ALU.mult)
            for half in range(2):
                d = 2 * qc + half
                nc.sync.dma_start(
                    a2a_src[b][d * P + h * 64 : d * P + (h + 1) * 64, :],
                    att[:, half * HTC : (half + 1) * HTC],
                )

        def attn_panel(b, qc, fillers, stride=5, start_kt=0):
            q0 = b * S + qc * 512
            qsl = slice(q0, q0 + 512)
            avqs = [
                avp.tile([65, 512], f32, tag="av", name=f"avq{b}{qc}{h}")
                for h in range(2)
            ]
            et = None
            for kt in range(NKT):
                ksl = slice(b * S + kt * P, b * S + (kt + 1) * P)
                lg = lgp.tile([P, 1024], f32, tag="lg", name="lg")
                nc.tensor.matmul(
                    lg[:, 0:512], qkvT[0:64, 1, ksl], qkvT[0:64, 0, qsl],
                    start=True, stop=True, tile_position=(0, 0),
                )
                nc.tensor.matmul(
                    lg[:, 512:1024], qkvT[64:128, 1, ksl], qkvT[64:128, 0, qsl],
                    start=True, stop=True, tile_position=(64, 0),
                )
                if kt % 2 == 0:
                    et = etp.tile([P, 2, 1024], f8, tag="et")
                # shift logits down so exp stays inside fp8e4 range
                # (softmax is shift-invariant; the ones-column sums shift too)
                nc.scalar.activation(
                    et[:, kt % 2, :], lg, AF.Exp, scale=1.0 / np.sqrt(HD),
                    bias=nbias_col,
                )
                if kt % 2 == 1:
                    jj = (b * NKT + kt) // 2
                    for h in range(2):
                        nc.tensor.matmul(
                            avqs[h],
                            vext[:, jj, :, h * 80 : h * 80 + 65],
                            et[:, :, h * 512 : (h + 1) * 512],
                            start=(kt == 1), stop=(kt == NKT - 1),
                            perf_mode=mybir.MatmulPerfMode.DoubleRow,
                        )
                if fillers and kt >= start_kt and kt % stride == stride - 1:
                    fillers.pop(0)()
            for h in range(2):
                epilogue(b, qc, h, avqs[h])

        with nc.named_scope("attn_b0"):
            for qc in range(NQC):
                attn_panel(0, qc, prep_quanta, stride=5)
            while prep_quanta:
                prep_quanta.pop(0)()

        with nc.named_scope("a2a0"):
            nc.gpsimd.collective_compute(
                "AllToAll", mybir.AluOpType.bypass,
                replica_groups=[core_ids],
                ins=[a2a_src[0][:]], outs=[a2a_dst[0][:]],
            )
        # DMAs consumed by the tail-0 pipeline that fills the b=1 window
        nc.sync.dma_start(
            attn_all[:, 0, :, :], a2a_dst[0].rearrange("(k p) t -> p k t", p=P)
        )
        nc.sync.dma_start(wprojT_sb, wprojT_d.rearrange("p (k e) -> p k e", k=KT))

        # ---- tail-0 pipeline as PE filler quanta for the b=1 window ----
        FQ = FF // 4
        NF = FF // P  # 32
        NQ = NF // 4  # 8
        w1t = {}
        ln2_state = {}

        def load_w1(b, q):
            w1t[q] = w1p.tile([P, KT, FQ], bf16, tag="w1", name=f"w1q{b}{q}")
            nc.sync.dma_start(w1t[q], w1_r[:, q, :, :])

        def proj_quantum(b, m):
            hsl = slice(b * HTC, (b + 1) * HTC)
            aa = attn_all[:, b, :, :]
            ps = psA.tile([P, HTC], f32, tag="a", name="projps")
            for kt in range(KT):
                nc.tensor.matmul(
                    ps, wprojT_sb[:, kt, m * P : (m + 1) * P], aa[:, kt, :],
                    start=(kt == 0), stop=(kt == KT - 1),
                )
            nc.vector.tensor_tensor(x1[:, m, hsl], ps, xcb_sb[:, m, hsl], ALU.add)
            if has_bproj:
                nc.vector.tensor_scalar(
                    x1[:, m, hsl], x1[:, m, hsl],
                    aux_sb[:, m : m + 1], None, ALU.add,
                )

        def ln2_stats_quantum(b, half):
            hsl = slice(b * HTC, (b + 1) * HTC)
            if half == 0:
                ln2_state["pmu"] = pmu = psA.tile([P, HTC], f32, tag="a", name="pmu2")
                ln2_state["psq"] = psq = psA.tile([P, HTC], f32, tag="a", name="psq2")
            else:
                pmu, psq = ln2_state["pmu"], ln2_state["psq"]
            xsqs = []
            for kt in range(half * 4, half * 4 + 4):
                xsq = work.tile([P, HTC], bf16, tag="xsq", bufs=4)
                nc.vector.tensor_tensor(xsq, x1[:, kt, hsl], x1[:, kt, hsl], ALU.mult)
                xsqs.append(xsq)
            for kt in range(half * 4, half * 4 + 4):
                nc.tensor.matmul(
                    pmu, ones128, x1[:, kt, hsl],
                    start=(kt == 0), stop=(kt == KT - 1),
                )
                nc.tensor.matmul(
                    psq, ones128, xsqs.pop(0), start=(kt == 0), stop=(kt == KT - 1)
                )

        def ln2_finish_quantum(b):
            hsl = slice(b * HTC, (b + 1) * HTC)
            pmu, psq = ln2_state.pop("pmu"), ln2_state.pop("psq")
            m1 = work.tile([P, HTC], f32, tag="m1")
            nc.vector.tensor_scalar_mul(m1, pmu, 1.0 / D)
            v1 = work.tile([P, HTC], f32, tag="v1")
            nc.vector.tensor_scalar_mul(v1, psq, 1.0 / D)
            m2 = work.tile([P, HTC], f32, tag="m2")
            nc.vector.tensor_tensor(m2, m1, m1, ALU.mult)
            nc.vector.tensor_tensor(v1, v1, m2, ALU.subtract)
            lnv = work.tile([P, HTC], f32, tag="m2")
            nc.scalar.activation(lnv, v1, AF.Ln, bias=eps_col)
            r2 = work.tile([P, HTC], f32, tag="v1")
            nc.scalar.activation(r2, lnv, AF.Exp, scale=-0.5)
            for kt in range(KT):
                nc.vector.tensor_tensor(
                    x1aug[:, kt, hsl], x1[:, kt, hsl], m1, ALU.subtract
                )
                nc.vector.tensor_tensor(
                    x1aug[:, kt, hsl], x1aug[:, kt, hsl], r2, ALU.mult
                )

        def mlp1_quantum(b, j, load_q=None):
            hsl = slice(b * HTC, (b + 1) * HTC)
            if load_q is not None:
                load_w1(b, load_q)
            w1h = w1t[j // NQ]
            msl = slice((j % NQ) * P, (j % NQ + 1) * P)
            ps = psA.tile([P, HTC], f32, tag="a", name="m1ps")
            for kt in range(KT):
                nc.tensor.matmul(
                    ps, w1h[:, kt, msl], x1aug[:, kt, hsl],
                    start=(kt == 0), stop=(kt == KT - 1),
                )
            bias_arg = aux_sb[:, 8 + j : 9 + j] if has_b1 else 0.0
            nc.scalar.activation(h2T[:, j, hsl], ps, AF.Relu, bias=bias_arg)

        def tail_front_quanta(b):
            qs = []
            if b == 0:
                qs.append(lambda: (load_w1(b, 0), load_w1(b, 1)))
                js = list(range(NF))
                loads = {0: 2, 8: 3}  # prefetch next quarters into the ring
            else:
                # quarters 2,3 are still resident from tail 0; consume them
                # first while 0,1 stream back in
                js = list(range(2 * NQ, NF)) + list(range(0, 2 * NQ))
                loads = {16: 0, 24: 1}
            for m in range(KT):
                qs.append(lambda m=m: proj_quantum(b, m))
            qs.append(lambda: ln2_stats_quantum(b, 0))
            qs.append(lambda: ln2_stats_quantum(b, 1))
            qs.append(lambda: ln2_finish_quantum(b))
            for j in js:
                qs.append(lambda j=j: mlp1_quantum(b, j, loads.get(j)))
            return qs

        tail0 = tail_front_quanta(0)

        with nc.named_scope("attn_b1"):
            attn_panel(1, 0, None)
            attn_panel(1, 1, None)
            attn_panel(1, 2, tail0, stride=1, start_kt=8)
            attn_panel(1, 3, tail0, stride=1)

        with nc.named_scope("a2a1"):
            nc.gpsimd.collective_compute(
                "AllToAll", mybir.AluOpType.bypass,
                replica_groups=[core_ids],
                ins=[a2a_src[1][:]], outs=[a2a_dst[1][:]],
            )
        with nc.named_scope("tail0_rest"):
            while tail0:
                tail0.pop(0)()

        attn_stack.close()

        # ============ phase 3: mlp2 + full b=1 tail =========================
        tail_stack = ExitStack()
        twork = tail_stack.enter_context(tc.tile_pool(name="twork", bufs=2))
        acc_pool = tail_stack.enter_context(
            tc.tile_pool(name="accp", bufs=4, space="PSUM")
        )
        w2_pool = tail_stack.enter_context(tc.tile_pool(name="w2p", bufs=1))

        w2t_tiles = {}

        def load_w2(b, q):
            w2t_tiles[q] = w2_pool.tile(
                [P, NQ, D], bf16, tag=f"w2{q}", name=f"w2q{b}{q}"
            )
            nc.sync.dma_start(w2t_tiles[q], w2_r[:, q, :, :])

        def tail_mlp2(b):
            hsl = slice(b * HTC, (b + 1) * HTC)
            with nc.named_scope(f"mlp2_{b}"):
                accs = [
                    acc_pool.tile([P, 512], f32, tag="acc", name=f"acc{b}{g}")
                    for g in range(4)
                ]

                def acc_sl(m):
                    return accs[m // 2][:, (m % 2) * HTC : (m % 2 + 1) * HTC]

                for kt2 in range(NF):
                    for m in range(KT):
                        # two d-tiles share one PSUM bank; only the first may
                        # issue start (it clears the whole bank), the second
                        # overwrites via cleared has_written bits
                        nc.tensor.matmul(
                            acc_sl(m), w2t_tiles[kt2 // NQ][:, kt2 % NQ, m * P : (m + 1) * P],
                            h2T[:, kt2, hsl],
                            start=(kt2 == 0 and m % 2 == 0),
                            stop=(kt2 == NF - 1),
                            skip_group_check=True,
                        )
                for m in range(KT):
                    ob = twork.tile([P, HTC], f32, tag="ob")
                    nc.vector.tensor_tensor(ob, acc_sl(m), x1[:, m, hsl], ALU.add)
                    if has_b2:
                        nc.vector.tensor_scalar(
                            ob, ob, aux_sb[:, 40 + m : 41 + m], None, ALU.add
                        )
                    nc.sync.dma_start(out_d[m * P : (m + 1) * P, hsl], ob)

        # w2 fully resident before the a2a1-dependent DMAs hit the queue
        # (stays resident for tail_mlp2(1) -- loaded only once)
        for q in range(4):
            load_w2(0, q)
        nc.sync.dma_start(
            attn_all[:, 1, :, :], a2a_dst[1].rearrange("(k p) t -> p k t", p=P)
        )
        nc.sync.dma_start(
            xcb_sb[:, :, HTC:TC],
            x_cb_d.rearrange("p (b k t) -> p b k t", b=B, k=KT)[:, 1, :, :],
        )
        tail_mlp2(0)

        with nc.named_scope("tail1"):
            for quantum in tail_front_quanta(1):
                quantum()
        tail_mlp2(1)
        tail_stack.close()

    nc.compile()
    return nc


def _prep_inputs(inputs):
    x = np.asarray(inputs["x"], np.float32)
    w_qkv = np.asarray(inputs["w_qkv"], np.float32)
    w_proj = np.asarray(inputs["w_proj"], np.float32)
    b_proj = np.asarray(inputs["b_proj"], np.float32)
    w1 = np.asarray(inputs["w1"], np.float32)
    b1 = np.asarray(inputs["b1"], np.float32)
    w2 = np.asarray(inputs["w2"], np.float32)
    b2 = np.asarray(inputs["b2"], np.float32)
    ln1_g = np.asarray(inputs["ln1_g"], np.float32)
    ln1_b = np.asarray(inputs["ln1_b"], np.float32)
    ln2_g = np.asarray(inputs["ln2_g"], np.float32)
    ln2_b = np.asarray(inputs["ln2_b"], np.float32)

    has_c1 = bool(np.any(ln1_b != 0))
    has_bproj = bool(np.any(b_proj != 0))
    has_c2 = bool(np.any(ln2_b != 0))
    has_b1 = bool(np.any(b1 != 0))
    has_b2 = bool(np.any(b2 != 0))
    flags = (has_c1, has_bproj, has_c2, has_b1, has_b2)

    xT = np.ascontiguousarray(x.reshape(T, D).T)  # [D, T] f32
    x_aug = np.zeros((DAUG, T), BF16)
    x_aug[:D] = xT.astype(BF16)
    # partition-major shuffle: [p, chunk, k, t] flattened per partition
    x_aug8 = np.ascontiguousarray(
        x_aug.reshape(KA, P, 8, 512).transpose(1, 2, 0, 3).reshape(P, -1)
    )

    wg = w_qkv * ln1_g[None, :]
    Se = wg.sum(axis=1)
    Ce = w_qkv @ ln1_b
    w1g = w1 * ln2_g[None, :]
    C2 = w1 @ ln2_b
    if np.any(C2 != 0):
        raise NotImplementedError("nonzero ln2_b not supported")

    FQ = FF // 4
    NQ = FF // P // 4
    w1_aug = (
        w1g.T.reshape(KT, P, 4, FQ).transpose(1, 2, 0, 3).reshape(P, -1).astype(BF16)
    )
    w2t = (
        w2.T.reshape(4, NQ, P, D).transpose(2, 0, 1, 3).reshape(P, -1).astype(BF16)
    )
    wprojT = (
        w_proj.T.reshape(KT, P, D).transpose(1, 0, 2).reshape(P, -1).astype(BF16)
    )

    in_maps = []
    for c in range(NCORES):
        rsl = slice(c * P, (c + 1) * P)
        wstack = np.concatenate([wg[rsl], wg[D:][rsl], wg[2 * D :][rsl]], axis=0)
        sstack = np.concatenate([Se[rsl], Se[D:][rsl], Se[2 * D :][rsl]])
        cstack = np.concatenate([Ce[rsl], Ce[D:][rsl], Ce[2 * D :][rsl]])
        wqkv_aug = np.zeros((DAUG, 3 * P), np.float32)
        wqkv_aug[:D] = wstack.T
        wqkv_aug[D] = sstack
        wqkv_aug = np.ascontiguousarray(
            wqkv_aug.reshape(KA, P, 3 * P).transpose(1, 0, 2).reshape(P, -1)
        ).astype(BF16)

        aux = np.zeros((P, 64), np.float32)
        aux[:, 0:8] = b_proj.reshape(KT, P).T
        aux[:, 8:40] = b1.reshape(FF // P, P).T
        aux[:, 40:48] = b2.reshape(KT, P).T
        aux[:, 48:51] = cstack.reshape(3, P).T

        own = np.r_[c * HTC : (c + 1) * HTC, S + c * HTC : S + (c + 1) * HTC]
        # [p, b, k, t] flattened per partition
        x_cb = np.ascontiguousarray(
            xT[:, own].reshape(KT, P, B, HTC).transpose(1, 2, 0, 3).reshape(P, -1)
        ).astype(BF16)

        in_maps.append(
            {
                "x_aug": x_aug8,
                "x_cb": x_cb,
                "wqkv_aug": wqkv_aug,
                "wprojT": wprojT,
                "w1_aug": w1_aug,
                "w2t": w2t,
                "aux": aux,
            }
        )
    return flags, in_maps


def _run(inputs, trace=False, trace_kwargs=None):
    from concourse.bass_utils import run_bass_kernel_spmd

    flags, in_maps = _prep_inputs(inputs)
    if flags not in _CACHE:
        _CACHE[flags] = _build_program(*flags)
    nc = _CACHE[flags]
    res = run_bass_kernel_spmd(
        nc, in_maps, list(range(NCORES)), trace=trace,
        **(trace_kwargs or {}),
    )
    outT = np.empty((D, T), np.float32)
    for c in range(NCORES):
        own = np.r_[c * HTC : (c + 1) * HTC, S + c * HTC : S + (c + 1) * HTC]
        outT[:, own] = res.results[c]["out_c"]
    out = np.ascontiguousarray(outT.T).reshape(B, S, D)
    return out, res


def kernel(**inputs):
    out, _ = _run(inputs, trace=False)
    return out
